# revision 1
# baseline (speedup 1.0000x reference)
"""Trainium2 Bass kernel for nn_CapsuleLayer (B=64, L=512, D=1024, C=32, O=64).

Strategy: data-parallel over batch across 8 NeuronCores (8 batch elements per
core), processed in 4 groups of 2. The PE engine queue is in-order, so the
projection of group g+1 is EMITTED INTERLEAVED into the routing of group g at
its semaphore-stall points (softmax / squash / W-build chains on ACT+DVE);
the PE then always has runnable work queued ahead of a stalled instruction.

Per core, per batch element:
  u_hatT[co, l] = fc_w.T @ xT (+ fc_b)      PE, weight tile shared across the
                                            group's batch elements (LDWEIGHTS
                                            amortization: loop m -> k -> i)
  u_hat[l, co]  = transpose(u_hatT)          PE transpose + DVE copy,
                                             interleaved per m-pair
  3 routing iterations, all on-chip:
    c_ij = softmax(b_ij) (no max-sub; logits are O(1))
    s_j  = diag-extract(c_ij.T @ u_hat)      PE cross-matmul, batches packed
                                             into PSUM partitions, bank-major
                                             with a 2-bank rotation
    v_j  = squash(s_j)                       ACT/DVE
    b_ij += u_hat . v_j                      PE: 16 accumulating matmuls with
                                             block-diag masked weights vs
                                             u_hatT; the delta transposes
                                             accumulate directly into a PSUM
                                             bank holding b_ij (softmax exp
                                             reads PSUM on the ACT engine)
"""

import contextlib
import ctypes
import sys
import types

import numpy as np
import ml_dtypes

B, L, D = 64, 512, 1024
C, O = 32, 64
CO = C * O                  # 2048
ITERS = 3
NCORES = 8
BPC = B // NCORES           # 8 batch elements per core
GB = 2                      # batch elements per routing group
NGRP = BPC // GB            # 4
P = 128
KD = D // P                 # 8 contraction chunks
MT = CO // P                # 16 m-tiles of u_hatT (= capsule pairs)
LT = L // P                 # 4 l-chunks
NBANK = CO // 512           # 4 cross-matmul column banks
SP = GB * C                 # s-cross partitions (64)

_BF16 = ml_dtypes.bfloat16

# ---------------------------------------------------------------------------
# NTFF profiling shim (used when tracing is requested by the test harness)
# ---------------------------------------------------------------------------


def _install_ntff_shim():
    if "antenv.axon_hooks" in sys.modules:
        return
    so_path = "/opt/axon/libaxon_pjrt.so"
    hook = None
    try:
        lib = ctypes.CDLL(so_path)
        if hasattr(lib, "axon_start_nrt_profile"):
            lib.axon_start_nrt_profile.argtypes = [
                ctypes.POINTER(ctypes.c_int64),
                ctypes.c_size_t,
            ]
            lib.axon_start_nrt_profile.restype = ctypes.c_int64
            lib.axon_stop_nrt_profile.argtypes = [ctypes.c_char_p]
            lib.axon_stop_nrt_profile.restype = ctypes.c_int64

            @contextlib.contextmanager
            def hook(output_dir, device_ids):
                import jax

                jax.devices()
                if device_ids:
                    ids = (ctypes.c_int64 * len(device_ids))(*device_ids)
                    rc = lib.axon_start_nrt_profile(ids, len(device_ids))
                else:
                    rc = lib.axon_start_nrt_profile(None, 0)
                if rc != 0:
                    raise RuntimeError(f"axon_start_nrt_profile rc={rc}")
                try:
                    yield
                finally:
                    n = lib.axon_stop_nrt_profile(str(output_dir).encode())
                    if n < 0:
                        raise RuntimeError(f"axon_stop_nrt_profile rc={n}")
    except OSError:
        pass
    mod = types.ModuleType("antenv.axon_hooks")
    mod.get_axon_ntff_profile_hook = lambda: hook
    mod.set_axon_ntff_profile_hook = lambda h: None
    sys.modules["antenv.axon_hooks"] = mod

    import concourse.bass_utils as bu

    bu.upload_artifacts = lambda tmpdir: tmpdir


# ---------------------------------------------------------------------------
# Kernel builder
# ---------------------------------------------------------------------------


def build_kernel():
    import concourse.bacc as bacc
    import concourse.tile as tile
    import concourse.mybir as mybir

    f32 = mybir.dt.float32
    bf16 = mybir.dt.bfloat16
    AF = mybir.ActivationFunctionType
    ALU = mybir.AluOpType
    AX = mybir.AxisListType

    nc = bacc.Bacc("TRN2", target_bir_lowering=False, debug=False)

    xt_d = nc.dram_tensor("xt", [BPC, D, L], bf16, kind="ExternalInput")
    w_d = nc.dram_tensor("w", [D, CO], bf16, kind="ExternalInput")
    ident_d = nc.dram_tensor("ident", [P, P], bf16, kind="ExternalInput")
    bias_d = nc.dram_tensor("bias_t", [P, MT], f32, kind="ExternalInput")
    identf_d = nc.dram_tensor("identf", [C, C], f32, kind="ExternalInput")
    identf128_d = nc.dram_tensor("identf128", [P, P], f32, kind="ExternalInput")
    mj_d = nc.dram_tensor("mj", [P, C], bf16, kind="ExternalInput")
    id64_d = nc.dram_tensor("id64", [P, O], bf16, kind="ExternalInput")
    m0u_d = nc.dram_tensor("m0u", [O, MT * C], bf16, kind="ExternalInput")
    m0l_d = nc.dram_tensor("m0l", [O, MT * C], bf16, kind="ExternalInput")
    maskx_d = nc.dram_tensor("mask_x", [P, C], f32, kind="ExternalInput")
    out_d = nc.dram_tensor("v", [BPC * C, O], f32, kind="ExternalOutput")

    with tile.TileContext(nc) as tc, contextlib.ExitStack() as glb:
        const_pool = glb.enter_context(tc.tile_pool(name="consts", bufs=1))
        w_pool = glb.enter_context(tc.tile_pool(name="w", bufs=KD))
        xt_pool = glb.enter_context(tc.tile_pool(name="xt", bufs=2 * GB * KD))
        ut_pool = glb.enter_context(tc.tile_pool(name="ut", bufs=2 * GB))
        u_pool = glb.enter_context(tc.tile_pool(name="u", bufs=2 * GB * LT))
        sm_pool = glb.enter_context(tc.tile_pool(name="sm", bufs=2))
        pp_mm = glb.enter_context(tc.tile_pool(name="ppmm", bufs=2, space="PSUM"))
        pp_tr = glb.enter_context(tc.tile_pool(name="pptr", bufs=2, space="PSUM"))
        ps_s_pool = glb.enter_context(tc.tile_pool(name="pss", bufs=2, space="PSUM"))
        ps_b_pool = glb.enter_context(tc.tile_pool(name="psb", bufs=1, space="PSUM"))
        ps_m_pool = glb.enter_context(tc.tile_pool(name="psm", bufs=1, space="PSUM"))

        # group-0 inputs go first on the Sync queue: the first projection
        # matmuls need xt(g0,i0) + w[0], not the whole 4MB of weights
        xt_sb = {}   # (g, i, k) -> [P, L] bf16

        def load_group(g):
            for i in range(GB):
                b = g * GB + i
                for k in range(KD):
                    t = xt_pool.tile([P, L], bf16, tag="xt",
                                     name=f"xt_g{g}_{i}_{k}")
                    nc.sync.dma_start(t[:], xt_d[b, k * P:(k + 1) * P, :])
                    xt_sb[g, i, k] = t

        load_group(0)

        # --- constants ---
        w_sb = []
        for k in range(KD):
            wt = w_pool.tile([P, CO], bf16, tag="w", name=f"w{k}")
            nc.sync.dma_start(wt[:], w_d[k * P:(k + 1) * P, :])
            w_sb.append(wt)
        ident = const_pool.tile([P, P], bf16, name="ident")
        nc.sync.dma_start(ident[:], ident_d[:])
        identf = const_pool.tile([C, C], f32, name="identf")
        nc.sync.dma_start(identf[:], identf_d[:])
        identf128 = const_pool.tile([P, P], f32, name="identf128")
        nc.sync.dma_start(identf128[:], identf128_d[:])
        mj = const_pool.tile([P, C], bf16, name="mj")
        nc.sync.dma_start(mj[:], mj_d[:])
        id64 = const_pool.tile([P, O], bf16, name="id64")
        nc.sync.dma_start(id64[:], id64_d[:])
        m0u = const_pool.tile([O, MT * C], bf16, name="m0u")
        nc.sync.dma_start(m0u[:], m0u_d[:])
        m0l = const_pool.tile([O, MT * C], bf16, name="m0l")
        nc.sync.dma_start(m0l[:], m0l_d[:])
        maskx = const_pool.tile([P, C], f32, name="maskx")
        nc.sync.dma_start(maskx[:], maskx_d[:])
        bias_sb = const_pool.tile([P, MT], f32, name="bias_sb")
        nc.sync.dma_start(bias_sb[:], bias_d[:])
        eps_sb = const_pool.tile([P, 1], f32, name="eps_sb")
        nc.vector.memset(eps_sb[:], 1e-8)

        tiles = {}   # g -> (UT, U)

        def alloc_group(g):
            UT = {}  # i -> [P, MT, L] bf16, partitions = co within m-chunk
            U = {}   # (i, lt) -> [P, CO] bf16, partitions = l chunk lt
            for i in range(GB):
                UT[i] = ut_pool.tile([P, MT, L], bf16, tag="ut",
                                     name=f"ut_g{g}_{i}")
                for lt in range(LT):
                    U[i, lt] = u_pool.tile([P, CO], bf16, tag="u",
                                           name=f"u_g{g}_{i}_{lt}")
            # per-(co-partition, batch, m) rowsum of u_hatT, accumulated for
            # free by the projection drains; seeds iteration 0 (c uniform)
            rsg = sm_pool.tile([P, GB, MT], f32, tag="rs", bufs=NGRP,
                               name=f"rs_g{g}")
            nc.vector.memset(rsg[:], 0.0)
            tiles[g] = (UT, U, rsg)

        def proj_gen(g):
            """Yield after emitting each m-step of group g's projection."""
            UT, U, rsg = tiles[g]
            ptr = {}
            for m in range(MT):
                mh = m % 2
                ps = {}
                for i in range(GB):
                    ps[i] = pp_mm.tile([P, 512], f32, tag="mm",
                                       name=f"ps_g{g}_{m}_{i}")
                for i in range(GB):
                    for k in range(KD):
                        nc.tensor.matmul(
                            ps[i][:],
                            w_sb[k][:, m * P:(m + 1) * P],
                            xt_sb[g, i, k][:],
                            start=(k == 0),
                            stop=(k == KD - 1),
                        )
                for i in range(GB):
                    # u_hatT = psum + bias (bias varies per partition=co)
                    nc.scalar.activation(
                        UT[i][:, m, :], ps[i][:], AF.Identity,
                        bias=bias_sb[:, m:m + 1],
                        accum_out=rsg[:, i, m:m + 1],
                    )
                    if mh == 0:
                        ptr[i] = pp_tr.tile([P, 2 * L], bf16, tag="tr",
                                            name=f"ptr_g{g}_{m}_{i}")
                    for lt in range(LT):
                        nc.tensor.matmul(
                            ptr[i][:, (mh * LT + lt) * P:
                                   (mh * LT + lt + 1) * P],
                            UT[i][:, m, lt * P:(lt + 1) * P],
                            ident[:],
                            is_transpose=True,
                            start=(mh == 0 and lt == 0),
                            stop=(mh == 1 and lt == LT - 1),
                        )
                    if mh == 1:
                        for mh2 in range(2):
                            for lt in range(LT):
                                nc.vector.tensor_copy(
                                    U[i, lt][:, (m - 1 + mh2) * P:
                                             (m + mh2) * P],
                                    ptr[i][:, (mh2 * LT + lt) * P:
                                           (mh2 * LT + lt + 1) * P],
                                )
                yield m

        def routing_gen(g):
            """Emit routing for group g as a generator, yielding at the
            semaphore-stall points (softmax / squash / W-build chains) so the
            driver can pump other emission there and keep the in-order PE
            queue fed."""
            UT, U, rsg = tiles[g]

            # b_ij lives in a PSUM bank, accumulated by the delta transposes
            ps_b = ps_b_pool.tile([P, GB, LT, C], f32, tag="bij",
                                  name=f"bij_g{g}")

            def s_pass_and_squash(it, c_get):
                # bank-major with a 2-bank psum rotation; extraction of
                # bank n overlaps the s-matmuls of bank n+1
                s01 = sm_pool.tile([SP, O], f32, tag="s01",
                                   name=f"s01_g{g}_{it}")
                s23 = sm_pool.tile([SP, O], f32, tag="s23",
                                   name=f"s23_g{g}_{it}")
                sps = []
                for n in range(NBANK):
                    ps_s = ps_s_pool.tile([SP, 512], f32, tag="ss",
                                          name=f"pss_g{g}_{it}_{n}")
                    for lt in range(LT):
                        for i in range(GB):
                            nc.tensor.matmul(
                                ps_s[i * C:(i + 1) * C, :],
                                c_get(i, lt),
                                U[i, lt][:, n * 512:(n + 1) * 512],
                                start=(lt == 0),
                                stop=(lt == LT - 1),
                                tile_position=(0, i * C),
                                skip_group_check=True,
                            )
                    tmpb = sm_pool.tile([SP, 512], f32, tag="tmpb",
                                        bufs=1, name=f"tmpb_g{g}_{it}_{n}")
                    nc.vector.tensor_tensor(
                        tmpb[:].rearrange("p (c o) -> p c o", c=8),
                        ps_s[:].rearrange("p (c o) -> p c o", c=8),
                        maskx[:SP, n * 8:(n + 1) * 8].unsqueeze(2)
                            .broadcast_to((SP, 8, O)),
                        ALU.mult,
                    )
                    sp = sm_pool.tile([SP, O], f32, tag=f"spart{n}", bufs=1,
                                      name=f"sp_g{g}_{it}_{n}")
                    nc.vector.tensor_reduce(
                        sp[:],
                        tmpb[:].rearrange("p (c o) -> p o c", c=8),
                        AX.X, ALU.add,
                    )
                    sps.append(sp)
                    if n == 1:
                        nc.vector.tensor_tensor(s01[:], sps[0][:], sps[1][:],
                                                ALU.add)
                    elif n == 3:
                        nc.vector.tensor_tensor(s23[:], sps[2][:], sps[3][:],
                                                ALU.add)
                # stall point: the squash chain waits on the extraction;
                # pumped work emitted here keeps its ACT ops ahead of the
                # squash in the scalar queue
                yield
                s_all = sm_pool.tile([SP, O], f32, tag="sall",
                                     name=f"sall_g{g}_{it}")
                nc.vector.tensor_tensor(s_all[:], s01[:], s23[:], ALU.add)
                return squash_emit(s_all[:], it)

            def squash_emit(s_src, it):
                # squash: v = s * sq/(1+sq)/sqrt(sq+1e-8)
                ssq = sm_pool.tile([SP, O], f32, tag="ssq", bufs=1,
                                   name=f"ssq_g{g}_{it}")
                sq = sm_pool.tile([SP, 1], f32, tag="sq",
                                  name=f"sq_g{g}_{it}")
                nc.scalar.activation(ssq[:], s_src, AF.Square,
                                     accum_out=sq[:])
                r1 = sm_pool.tile([SP, 1], f32, tag="r1",
                                  name=f"r1_g{g}_{it}")
                nc.scalar.activation(r1[:], sq[:], AF.Sqrt, bias=eps_sb[:SP])
                r2 = sm_pool.tile([SP, 1], f32, tag="r2",
                                  name=f"r2_g{g}_{it}")
                nc.vector.scalar_tensor_tensor(
                    r2[:], sq[:], 1.0, r1[:], ALU.add, ALU.mult,
                )
                rr = sm_pool.tile([SP, 1], f32, tag="rr",
                                  name=f"rr_g{g}_{it}")
                nc.vector.reciprocal(rr[:], r2[:])
                v_all = sm_pool.tile([SP, O], f32, tag="vall",
                                     name=f"vall_g{g}_{it}")
                # v = (s * sq) * (1 / ((1+sq) sqrt(sq+eps)))
                nc.vector.tensor_scalar(
                    v_all[:], s_src, sq[:], rr[:], ALU.mult, ALU.mult,
                )
                return v_all

            c_cur = [None]

            def c_cur_get(i, lt):
                return c_cur[0][:, i * LT + lt, :]

            for it in range(ITERS - 1):
                if it == 0:
                    # c is uniform at iteration 0, so s0 = rowsum(u_hat)/C;
                    # the rowsums were accumulated for free by the drains.
                    # tmp[p, i, 2m+j] = rs[p, i, m] * (p//64 == j)/C, then
                    # s0[(i,c), o] = sum_p tmp[p, i, c] * (p%64 == o)
                    tmp0 = sm_pool.tile([P, GB, MT, 2], bf16, tag="s0t",
                                        name=f"s0t_g{g}")
                    nc.vector.tensor_tensor(
                        tmp0[:],
                        rsg[:].unsqueeze(3).broadcast_to((P, GB, MT, 2)),
                        mj[:].rearrange("p (m j) -> p m j", m=MT)
                            .unsqueeze(1).broadcast_to((P, GB, MT, 2)),
                        ALU.mult,
                    )
                    ps_s0 = ps_s_pool.tile([SP, O], f32, tag="ss",
                                           name=f"pss0_g{g}")
                    for i in range(GB):
                        nc.tensor.matmul(
                            ps_s0[i * C:(i + 1) * C, :],
                            tmp0[:, i, :, :],
                            id64[:],
                            start=True,
                            stop=True,
                            tile_position=(0, i * C),
                            skip_group_check=True,
                        )
                    yield
                    v_all = squash_emit(ps_s0[:], it)
                else:
                    v_all = yield from s_pass_and_squash(it, c_cur_get)
                yield  # cover the squash chain stall

                # vT_all[o, (i,c)] = v_all[(i,c), o]
                ps_v = ps_m_pool.tile([O, SP], f32, tag="misc",
                                      name=f"psv_g{g}_{it}")
                nc.tensor.transpose(ps_v[:], v_all[:], identf128[:SP, :SP])

                # W[(j,o), (i, t, c)] = v_i[2t+j, o] * (c == 2t+j)
                W = sm_pool.tile([P, GB, MT, C], bf16, tag="W",
                                 bufs=1, name=f"W_g{g}_{it}")
                pv = ps_v[:].rearrange("o (i c) -> o i c", i=GB)
                nc.vector.tensor_tensor(
                    W[:O],
                    pv[:, :, ::2].unsqueeze(3).broadcast_to((O, GB, MT, C)),
                    m0u[:].rearrange("p (t c) -> p t c", t=MT)
                        .unsqueeze(1).broadcast_to((O, GB, MT, C)),
                    ALU.mult,
                )
                nc.vector.tensor_tensor(
                    W[O:],
                    pv[:, :, 1::2].unsqueeze(3).broadcast_to((O, GB, MT, C)),
                    m0l[:].rearrange("p (t c) -> p t c", t=MT)
                        .unsqueeze(1).broadcast_to((O, GB, MT, C)),
                    ALU.mult,
                )
                yield  # cover the W-build chain stall

                for i in range(GB):
                    # deltaT[c, l] = sum_t W_t.T @ u_hatT_t
                    ps_d = ps_m_pool.tile([C, 512], f32, tag="misc",
                                          name=f"psd_g{g}_{it}_{i}")
                    for t in range(MT):
                        nc.tensor.matmul(
                            ps_d[:],
                            W[:, i, t, :],
                            UT[i][:, t, :],
                            start=(t == 0),
                            stop=(t == MT - 1),
                        )
                    ds = sm_pool.tile([C, 512], f32, tag="ds", bufs=1,
                                      name=f"ds_g{g}_{it}_{i}")
                    nc.scalar.copy(ds[:], ps_d[:])
                    # b_ij accumulates in PSUM via the delta transposes.
                    # start=True clears has_written for the whole 2KB zero
                    # region, so it must be issued exactly once per group.
                    for lt in range(LT):
                        nc.tensor.matmul(
                            ps_b[:, i, lt, :],
                            ds[:, lt * P:(lt + 1) * P],
                            identf[:],
                            is_transpose=True,
                            start=(it == 0 and i == 0 and lt == 0),
                            stop=(it == ITERS - 2 and i == GB - 1
                                  and lt == LT - 1),
                            skip_group_check=True,
                        )

                # cover the softmax chain stall; emitting pumped work here
                # (not at loop top) keeps its ACT drains ahead of the exp in
                # the scalar queue
                yield

                # per-batch softmax over capsules for the next iteration
                c_next = sm_pool.tile([P, GB * LT, C], bf16, tag="cij",
                                      name=f"cij_g{g}_{it + 1}")
                cexp = sm_pool.tile([P, GB * LT, C], f32, tag="cexp",
                                    bufs=1, name=f"cexp_g{g}_{it + 1}")
                csum = sm_pool.tile([P, GB * LT], f32, tag="csum",
                                    name=f"csum_g{g}_{it + 1}")
                crec = sm_pool.tile([P, GB * LT], f32, tag="crec",
                                    name=f"crec_g{g}_{it + 1}")
                nc.scalar.activation(
                    cexp[:].rearrange("p q c -> p (q c)"),
                    ps_b[:].rearrange("p i q c -> p (i q c)"),
                    AF.Exp,
                )
                nc.vector.tensor_reduce(csum[:], cexp[:], AX.X, ALU.add)
                nc.vector.reciprocal(crec[:], csum[:])
                nc.vector.tensor_tensor(
                    c_next[:], cexp[:],
                    crec[:].unsqueeze(2).broadcast_to((P, GB * LT, C)),
                    ALU.mult,
                )
                c_cur[0] = c_next

            # last iteration: no b_ij update
            yield  # cover the softmax chain stall
            v_all = yield from s_pass_and_squash(ITERS - 1, c_cur_get)
            nc.scalar.dma_start(out_d[g * SP:(g + 1) * SP, :], v_all[:])

        # --- pipelined emission: routing(g) is the main stream; its stall
        # points are filled first with proj(g+1) steps, then with the head
        # of routing(g+1) itself, which the next phase continues.
        import itertools

        alloc_group(0)
        for _ in proj_gen(0):
            pass
        rgens = {}
        for g in range(NGRP):
            if g + 1 < NGRP:
                load_group(g + 1)
                alloc_group(g + 1)
                rgens[g + 1] = routing_gen(g + 1)
                feed = itertools.chain(proj_gen(g + 1), rgens[g + 1])
            else:
                feed = iter(())
            # 10 yields x 2 pumps cover the 16 proj steps by yield 8; the
            # last 4 pumps pre-emit the head of routing(g+1) (through its
            # it0 delta), whose remainder continues as the next main. The
            # budget must not let a pre-emitted segment's single-buffer
            # pool ring (psb/psm/sm tags) wait on work this group has not
            # emitted yet — that would deadlock the in-order queues; at
            # 2/yield the head pulls land on yields 9-10, after this
            # group's it1 delta and softmax are already emitted, which
            # makes exactly 4 head pulls safe.
            budget = MT + 2
            _STOP = object()
            for _ in rgens.setdefault(g, routing_gen(g)):
                for _ in range(2):
                    if budget <= 0 or next(feed, _STOP) is _STOP:
                        break
                    budget -= 1

    nc.compile()
    return nc


_NC_CACHE = None


def _get_nc():
    global _NC_CACHE
    if _NC_CACHE is None:
        _NC_CACHE = build_kernel()
    return _NC_CACHE


def _make_consts():
    ident = np.eye(P, dtype=_BF16)
    identf = np.eye(C, dtype=np.float32)
    identf128 = np.eye(P, dtype=np.float32)
    mj = (((np.arange(P)[:, None] // O) == (np.arange(C)[None, :] % 2))
          .astype(np.float32) / C).astype(_BF16)
    id64 = np.tile(np.eye(O, dtype=_BF16), (2, 1))
    cc = np.arange(C)
    tt = np.arange(MT)
    m0u = np.broadcast_to(
        (cc[None, :] == 2 * tt[:, None]).astype(_BF16).reshape(1, MT * C),
        (O, MT * C)).copy()
    m0l = np.broadcast_to(
        (cc[None, :] == 2 * tt[:, None] + 1).astype(_BF16).reshape(1, MT * C),
        (O, MT * C)).copy()
    maskx = (cc[None, :] == (np.arange(P) % C)[:, None]).astype(np.float32)
    return ident, identf, identf128, mj, id64, m0u, m0l, maskx


def kernel(inputs, fc_w, fc_b, _trace=False):
    from concourse.bass_utils import run_bass_kernel_spmd

    if _trace:
        _install_ntff_shim()

    nc = _get_nc()

    ident, identf, identf128, mj, id64, m0u, m0l, maskx = _make_consts()
    w_bf = np.asarray(fc_w, dtype=np.float32).astype(_BF16)
    bias_t = np.ascontiguousarray(
        np.asarray(fc_b, dtype=np.float32).reshape(MT, P).T
    )
    xt_all = np.ascontiguousarray(
        np.asarray(inputs, dtype=np.float32).transpose(0, 2, 1)
    ).astype(_BF16)

    in_maps = []
    for core in range(NCORES):
        in_maps.append({
            "xt": xt_all[core * BPC:(core + 1) * BPC],
            "w": w_bf,
            "bias_t": bias_t,
            "ident": ident,
            "identf": identf,
            "identf128": identf128,
            "mj": mj,
            "id64": id64,
            "m0u": m0u,
            "m0l": m0l,
            "mask_x": maskx,
        })

    res = run_bass_kernel_spmd(
        nc, in_maps, core_ids=list(range(NCORES)), trace=_trace,
    )
    out = np.concatenate(
        [res.results[core]["v"].reshape(BPC, C, O) for core in range(NCORES)],
        axis=0,
    )
    if _trace:
        kernel.last_exec_time_ns = res.exec_time_ns
        kernel.last_results = res
    return out

